# revision 59
# baseline (speedup 1.0000x reference)
"""EquivariantEvolution kernel for 8 Trainium2 NeuronCores (Bass/Tile).

Math (per sample):
    alpha = W_se2 silu(W_se1 z + b_se1) + b_se2            # [NG=8]
    A     = sum_g alpha_g G_g                              # [32, 32]
    z_t   = (I + A + A^2/2 + A^3/6 + A^4/24) z             # order-4 Taylor
    h1    = W1 z_t + b1                                    # [128]
    out   = sigmoid(|h1| + eps) * (W2 h1) + b2             # gate commuted past W2

Device strategy (pure batch data-parallel, feature-major [feat, samples]):
  * Horner: v <- z + (1/k) A v.  A v as y[(g,i),b] = alpha_g[b] v_i[b]
    (elementwise outer product) contracted by two K=128 matmuls whose
    lhsT is pre-replicated 4x along M so outputs land ready for the next
    elementwise step.  The +z fold is a K=32 matmul row-packed 2x via
    tile_position so pairs run concurrently in the PE array.
  * Everything the PE touches is bf16 (FWL weight loads, half DMA);
    all accumulation stays f32 in PSUM.
  * The norm-squared reduction for all 16 tiles accumulates into ONE
    PSUM bank (4 col-groups x 4 rows), so the sqrt/tanh run once per
    kernel (2 ACT table switches total).
  * Phase B: gate = 0.5*tanh+0.5 is broadcast to 4 tiles at once by a
    single 0/0.5 matmul; W2 h1 is col-packed 4x (M=32 each); one DVE
    multiply + one 4-strip DMA store finishes 4 tiles.
  * No HAM warm-up spam: a short burst of zero matmuls during the
    initial parameter DMAs brings the clock to K=8/8; after that the
    real matmul stream is dense enough to keep it there.
"""

import os
import sys

import numpy as np

for _p in ("/opt/trn_rl_repo", "/root/.axon_site/_ro/trn_rl_repo"):
    if os.path.isdir(_p) and _p not in sys.path:
        sys.path.insert(0, _p)

import concourse.bass as bass
import concourse.mybir as mybir
import concourse.tile as tile
from concourse.bass_utils import run_bass_kernel_spmd

B, D, H, NG = 65536, 32, 128, 8
NCORES = 8
BC = B // NCORES          # samples per core
BT = 512                  # samples per tile (PSUM bank width in f32)
GS = 4                    # tiles per group
EPS = 1e-6
F32 = mybir.dt.float32
F32R = mybir.dt.float32r
BF16 = mybir.dt.bfloat16
AF = mybir.ActivationFunctionType

# whether the z-path outer product (all-SBUF operands) runs on gpsimd
YCAT0_ON_GPSIMD = True


def _split_multi_waits(nc, max_waits=1):
    """This toolchain's walrus rejects >1 sync-wait on an instruction
    ("Too many sync wait commands"); hoist extra waits onto preceding
    same-engine NOPs (in-order engines make this semantics-preserving)."""
    n_new = 0
    for f in nc.m.functions:
        for bb in f.blocks:
            out = []
            for ins in bb.instructions:
                si = getattr(ins, "sync_info", None)
                if si is not None and si.on_wait and len(si.on_wait) > max_waits:
                    waits = list(si.on_wait)
                    chunks = [waits[i:i + max_waits] for i in range(0, len(waits), max_waits)]
                    for ci, ch in enumerate(chunks[:-1]):
                        nop = mybir.InstNoOp(
                            name=f"{ins.name}-wsplit{ci}",
                            engine=ins.engine,
                            sync_info=mybir.SyncInfo(on_wait=ch, on_update=[]),
                            bass_nofuse=True,
                        )
                        out.append(nop)
                        n_new += 1
                    ins.sync_info = mybir.SyncInfo(on_wait=chunks[-1], on_update=si.on_update)
                out.append(ins)
            bb.instructions[:] = out
    return n_new


# DRAM parameters: name -> (shape, dtype).  All matmul operands bf16.
_PARAM_SHAPES = {
    "LT_h4": ([H, H], BF16),      # W_se1^T tiled 4x along partitions
    "LT_At": ([H, H], BF16),      # W_se2[0:4] replicated 32x over M
    "LT_Ab": ([H, H], BF16),      # W_se2[4:8]
    "Bse1": ([H, 1], F32),
    "Bse2t": ([H, 1], F32),
    "Bse2b": ([H, 1], F32),
    "LT_t4": ([H, H], BF16), "LT_b4": ([H, H], BF16),
    "LT_t3": ([H, H], BF16), "LT_b3": ([H, H], BF16),
    "LT_t2": ([H, H], BF16), "LT_b2": ([H, H], BF16),
    "LT_t1": ([H, H], BF16), "LT_b1": ([H, H], BF16),   # W1-folded k=1 step
    "LT_z4": ([H, H], BF16),      # I32 tiled (4,4): row-packable +z fold
    "LT_w1z4": ([H, H], BF16),    # W1^T tiled 4x along partitions
    "B1": ([H, 1], F32),
    "onsq4": ([H, H], BF16),      # norm-sq row-select weights (variant r at cols 32r)
    "E4": ([H, 4 * H], BF16),     # 0.5-scaled gate broadcast, one [H,H] block per group
    "LT_w2c": ([H, H], BF16),     # W2^T tiled 4x along M (col-packable)
    "B2r": ([H, 1], F32),         # b2 tiled 4x along partitions
}


def _build_program(bc: int, zero_bias: bool, sim_safe: bool = False, split_waits: bool = True):
    nt = bc // BT
    ngrp = nt // GS
    nc = bass.Bass()

    zT = nc.declare_dram_parameter("zT", [D, bc], BF16, isOutput=False)
    params = {
        name: nc.declare_dram_parameter(name, shape, dt, isOutput=False)
        for name, (shape, dt) in _PARAM_SHAPES.items()
    }
    ngrp_ = (bc // BT) // GS
    out4 = nc.declare_dram_parameter("out4", [H, ngrp_ * BT], F32, isOutput=True)

    with tile.TileContext(nc) as tc:
        with (
            tc.tile_pool(name="consts", bufs=1) as consts,
            tc.tile_pool(name="zv4", bufs=3) as zv4_pool,
            tc.tile_pool(name="hs", bufs=3) as hs_pool,
            tc.tile_pool(name="acat", bufs=7) as acat_pool,   # [H,2,2,BT] pair tiles
            tc.tile_pool(name="ycat0", bufs=6) as ycat0_pool,  # long-lived z-products
            tc.tile_pool(name="ycat", bufs=5) as ycat_pool,   # short-lived step products
            tc.tile_pool(name="sq", bufs=5) as sq_pool,
            tc.tile_pool(name="h1s", bufs=nt) as h1s_pool,
            tc.tile_pool(name="gate", bufs=6) as gate_pool,
            tc.tile_pool(name="og", bufs=3) as og_pool,
            # PSUM: hp(1) + ap(1x2) + pv(2x2) + nsq(1) = 8 banks
            tc.tile_pool(name="hp", bufs=1, space=bass.MemorySpace.PSUM) as hp_pool,
            tc.tile_pool(name="ap", bufs=1, space=bass.MemorySpace.PSUM) as ap_pool,
            tc.tile_pool(name="pv", bufs=2, space=bass.MemorySpace.PSUM) as pv_pool,
            tc.tile_pool(name="psn", bufs=1, space=bass.MemorySpace.PSUM) as psn_pool,
        ):
            # ---- constants into SBUF (sync queue only: the scalar queue
            # would block the ACT FIFO, gpsimd carries the z loads) ----
            ct = {}
            qs = (nc.sync, nc.scalar, nc.sync, nc.gpsimd)
            for name, (shape, dt) in _PARAM_SHAPES.items():
                t = consts.tile(shape, dt, name=f"c_{name}")
                nc.sync.dma_start(t[:], params[name][:])
                ct[name] = t
            half_b = consts.tile([H, 1], F32, name="half_b")
            nc.vector.memset(half_b[:], 0.5)
            zero_b = consts.tile([H, 1], F32, name="zero_b")
            nc.vector.memset(zero_b[:], 0.0)
            tanh_b = consts.tile([H, 1], F32, name="tanh_b")
            nc.vector.memset(tanh_b[:], 0.5 * EPS)

            # ---- HAM warm filler: zero-matmuls accumulated into the live
            # norm-sq bank (wscr is all zeros, so they add exactly 0).  Keeps
            # the PE clock gate at K=8/8 wherever the real stream thins,
            # without needing a PSUM bank of their own. ----
            wscr = consts.tile([H, BT], BF16, name="wscr")
            nc.vector.memset(wscr[:], 0.0)
            nsq_cur = [psn_pool.tile([H, BT], F32, name="nsq_ps", tag="nsq")]

            def warm(n, cols=BT):
                for _ in range(n):
                    nc.tensor.matmul(nsq_cur[0][:, 0:cols], wscr[:, 0:H],
                                     wscr[:, 0:cols],
                                     start=False, stop=False, skip_group_check=True)

            warm(18)

            taylor = [
                (ct["LT_t4"], ct["LT_b4"]),
                (ct["LT_t3"], ct["LT_b3"]),
                (ct["LT_t2"], ct["LT_b2"]),
            ]

            h1s_tiles = []
            PAIRS = (0, 2)

            def emit_loads(g):
                zv4 = zv4_pool.tile([H, GS, BT], BF16, name="zv4")
                for s in range(4):
                    nc.gpsimd.dma_start(
                        zv4[32 * s:32 * (s + 1), :, :],
                        zT[:, bass.ts(g, GS * BT)],
                    )
                return zv4

            def emit_extractor(zv4, g=99):
                """alpha for 4 tiles; returns per-pair acat/ycat0 [H,2,2,BT]."""
                acats, ycats = {}, {}
                for jp in PAIRS:
                    acat = acat_pool.tile([H, 2, 2, BT], BF16, name="acat")
                    for jj in range(2):
                        j = jp + jj
                        hp = hp_pool.tile([H, BT], F32, name="hp", tag="hp")
                        nc.tensor.matmul(hp[:], ct["LT_h4"][0:D, :], zv4[0:D, j, :],
                                         start=True, stop=True)
                        hs = hs_pool.tile([H, BT], BF16, name="hs")
                        if sim_safe:
                            sg = hs_pool.tile([H, BT], F32, name="sg")
                            nc.scalar.activation(sg[:], hp[:], AF.Sigmoid, bias=ct["Bse1"][:])
                            hx = hs_pool.tile([H, BT], F32, name="hx")
                            nc.scalar.activation(hx[:], hp[:], AF.Identity, bias=ct["Bse1"][:])
                            nc.vector.tensor_tensor(hs[:], sg[:], hx[:], mybir.AluOpType.mult)
                        else:
                            nc.scalar.activation(hs[:], hp[:], AF.Silu, bias=ct["Bse1"][:])

                        ap = ap_pool.tile([H, 2, BT], F32, name="ap", tag="ap")
                        nc.tensor.matmul(ap[:, 0, :], ct["LT_At"][:], hs[:],
                                         start=True, stop=True)
                        nc.tensor.matmul(ap[:, 1, :], ct["LT_Ab"][:], hs[:],
                                         start=True, stop=True)
                        if zero_bias:
                            nc.scalar.activation(acat[:, :, jj, :], ap[:], AF.Identity)
                        else:
                            nc.scalar.activation(acat[:, 0, jj, :], ap[:, 0, :], AF.Identity,
                                                 bias=ct["Bse2t"][:])
                            nc.scalar.activation(acat[:, 1, jj, :], ap[:, 1, :], AF.Identity,
                                                 bias=ct["Bse2b"][:])
                    acats[jp] = acat

                    ycat = ycat0_pool.tile([H, 2, 2, BT], BF16, name="ycat0")
                    # ramp: DVE is idle for the first two groups; steady state
                    # keeps the z-path product off the loaded DVE
                    eng = nc.gpsimd if (YCAT0_ON_GPSIMD and g >= 2) else nc.vector
                    eng.tensor_tensor(
                        ycat[:], acat[:],
                        zv4[:, None, jp:jp + 2, :].broadcast_to([H, 2, 2, BT]),
                        mybir.AluOpType.mult,
                    )
                    ycats[jp] = ycat
                return acats, ycats

            def emit_taylor(zv4, acats, ycats):
                """Horner k=4,3,2 then W1-fused k=1; h1s/sq per tile."""
                for lt_top, lt_bot in taylor:
                    for jp in PAIRS:
                        pv2 = pv_pool.tile([H, 2, BT], F32, name="pv2", tag="pv")
                        for jj in range(2):
                            j = jp + jj
                            nc.tensor.matmul(
                                pv2[:, jj, :], ct["LT_z4"][bass.ts(j, 32), :],
                                zv4[bass.ts(j, 32), j, :],
                                start=True, stop=False, skip_group_check=True,
                                tile_position=(32 * j, 0),
                            )
                        for jj in range(2):  # tops then bots: adjacent same-weight MMs
                            nc.tensor.matmul(pv2[:, jj, :], lt_top[:],
                                             ycats[jp][:, 0, jj, :],
                                             start=False, stop=False, skip_group_check=True)
                        for jj in range(2):
                            nc.tensor.matmul(pv2[:, jj, :], lt_bot[:],
                                             ycats[jp][:, 1, jj, :],
                                             start=False, stop=True, skip_group_check=True)
                        ycat = ycat_pool.tile([H, 2, 2, BT], BF16, name="ycat")
                        nc.vector.tensor_tensor(
                            ycat[:], acats[jp][:],
                            pv2[:, None, :, :].broadcast_to([H, 2, 2, BT]),
                            mybir.AluOpType.mult,
                        )
                        ycats[jp] = ycat
                        warm(1)

                sqs = {}
                for jp in PAIRS:
                    h1p2 = pv_pool.tile([H, 2, BT], F32, name="h1p2", tag="pv")
                    for jj in range(2):
                        j = jp + jj
                        nc.tensor.matmul(
                            h1p2[:, jj, :], ct["LT_w1z4"][bass.ts(j, 32), :],
                            zv4[bass.ts(j, 32), j, :],
                            start=True, stop=False, skip_group_check=True,
                            tile_position=(32 * j, 0),
                        )
                    for jj in range(2):
                        nc.tensor.matmul(h1p2[:, jj, :], ct["LT_t1"][:],
                                         ycats[jp][:, 0, jj, :],
                                         start=False, stop=False, skip_group_check=True)
                    for jj in range(2):
                        nc.tensor.matmul(h1p2[:, jj, :], ct["LT_b1"][:],
                                         ycats[jp][:, 1, jj, :],
                                         start=False, stop=True, skip_group_check=True)
                    for jj in range(2):  # sq first: it feeds the gate critical path
                        j = jp + jj
                        sq = sq_pool.tile([H, BT], BF16, name="sq")
                        nc.scalar.activation(sq[:], h1p2[:, jj, :], AF.Square,
                                             bias=ct["B1"][:])
                        sqs[j] = sq
                    for jj in range(2):
                        h1s = h1s_pool.tile([H, BT], BF16, name="h1s")
                        nc.scalar.activation(h1s[:], h1p2[:, jj, :], AF.Identity,
                                             bias=ct["B1"][:])
                        h1s_tiles.append(h1s)
                    warm(1)
                return sqs

            def emit_nsq(g, sqs, nsq_ps):
                # norm-sq of tile 4g+j -> col-group j, row g%2 (4 col-packed MMs);
                # each nsq bank covers two groups.
                for j in range(GS):
                    nc.tensor.matmul(
                        nsq_ps[bass.ts(j, 32), :], ct["onsq4"][:, bass.ts(g % 2, 32)],
                        sqs[j][:],
                        start=(g % 2 == 0), stop=(g % 2 == 1), skip_group_check=True,
                        tile_position=(0, 32 * j),
                    )

            def emit_gate(nsq_ps):
                """batched sqrt + tanh for the two groups in one nsq bank"""
                rt = gate_pool.tile([H, BT], F32, name="rt")
                nc.scalar.activation(rt[:], nsq_ps[:], AF.Sqrt, bias=zero_b[:])
                t_all = gate_pool.tile([H, BT], BF16, name="t_all")
                # sigmoid(norm + eps) = 0.5 tanh(0.5 norm + eps/2) + 0.5
                nc.scalar.activation(t_all[:], rt[:], AF.Tanh, scale=0.5, bias=tanh_b[:])
                return t_all

            def emit_w2(gpair, pool):
                """gate-independent W2 h1 for two groups into one pair tile"""
                wpp = pool.tile([H, 2, BT], F32, name="wp2",
                                tag="ap" if pool is ap_pool else "pv")
                for gi in range(2):
                    for j in range(GS):  # col-packed W2 h1, M=32 each
                        nc.tensor.matmul(
                            wpp[bass.ts(j, 32), gi, :],
                            ct["LT_w2c"][:, bass.ts(j, 32)],
                            h1s_tiles[GS * (gpair + gi) + j][:],
                            start=True, stop=True, skip_group_check=True,
                            tile_position=(0, 32 * j),
                        )
                return wpp

            def emit_gate_b(g, t_all, wp):
                """gate-dependent part: broadcast, multiply, store"""
                trp = hp_pool.tile([H, BT], F32, name="trp", tag="hp")
                nc.tensor.matmul(trp[:], ct["E4"][:, bass.ts(g, H)], t_all[:],
                                 start=True, stop=True, skip_group_check=True)
                t2g = gate_pool.tile([H, BT], BF16, name="t2g")
                nc.scalar.activation(t2g[:], trp[:], AF.Identity, bias=half_b[:])
                og = og_pool.tile([H, BT], F32, name="og")
                nc.vector.tensor_tensor(og[:], wp, t2g[:], mybir.AluOpType.mult)
                if not zero_bias:
                    og2 = og_pool.tile([H, BT], F32, name="og2")
                    nc.scalar.activation(og2[:], og[:], AF.Identity, bias=ct["B2r"][:])
                    og = og2
                qs[g % 2].dma_start(out4[:, bass.ts(g, BT)], og[:])

            # ===== main schedule: pipelined phase A, gate split in halves so
            # phase B of groups 0-1 overlaps phase A of groups 2-3 =====
            from collections import deque
            pipe = deque()
            for g in range(min(2, ngrp)):
                zv4_g = emit_loads(g)
                pipe.append((zv4_g,) + emit_extractor(zv4_g, g))
            nsq_ps = None
            t_allA = None
            for g in range(ngrp):
                zv4_g, acats_g, ycats_g = pipe.popleft()
                if g + 2 < ngrp:
                    zv4_n = emit_loads(g + 2)
                    pipe.append((zv4_n,) + emit_extractor(zv4_n, g + 2))
                sqs = emit_taylor(zv4_g, acats_g, ycats_g)
                if g == 2:
                    nsq_cur[0] = psn_pool.tile([H, BT], F32, name="nsq_ps", tag="nsq")
                nsq_ps = nsq_cur[0]
                emit_nsq(g, sqs, nsq_ps)
                if g == 1:
                    t_allA = emit_gate(nsq_ps)
                elif g == 2:
                    wpA = emit_w2(0, ap_pool)  # ap pool is free after the last extractor
                    emit_gate_b(0, t_allA, wpA[:, 0, :])
                elif g == 3:
                    emit_gate_b(1, t_allA, wpA[:, 1, :])
                    wpB = emit_w2(2, pv_pool)  # pv pool free once phase A drains
                    t_allB = emit_gate(nsq_ps)
                    emit_gate_b(2, t_allB, wpB[:, 0, :])
                    emit_gate_b(3, t_allB, wpB[:, 1, :])

    if split_waits:
        _split_multi_waits(nc)
    return nc


def _host_params(G, W_se1, b_se1, W_se2, b_se2, W1, b1, W2, b2, nt):
    import ml_dtypes
    f = np.float32
    bf = ml_dtypes.bfloat16
    ngrp = nt // GS
    G = np.asarray(G, f)
    Gflat = np.transpose(G, (0, 2, 1)).reshape(NG * D, D)  # [(g,i), j] = G[g,j,i]
    W1G = Gflat @ np.asarray(W1, f).T                      # [(g,i), m]
    I32 = np.eye(D, dtype=f)

    onsq = np.zeros((H, 4, 32), f)
    for r in range(4):
        onsq[:, r, r] = 1.0
    E4 = np.zeros((H, ngrp, H), f)
    for g in range(ngrp):
        for r in range(GS):
            E4[32 * r + (g % 2), g, 32 * r:32 * (r + 1)] = 0.5

    p = {
        "LT_h4": np.tile(np.asarray(W_se1, f).T, (4, 1)),
        "LT_At": np.repeat(np.asarray(W_se2, f).T[:, 0:4], 32, axis=1),
        "LT_Ab": np.repeat(np.asarray(W_se2, f).T[:, 4:8], 32, axis=1),
        "Bse1": np.asarray(b_se1, f).reshape(H, 1),
        "Bse2t": np.repeat(np.asarray(b_se2, f)[0:4], 32).reshape(H, 1),
        "Bse2b": np.repeat(np.asarray(b_se2, f)[4:8], 32).reshape(H, 1),
        "LT_z4": np.tile(I32, (4, 4)),
        "LT_w1z4": np.tile(np.asarray(W1, f).T, (4, 1)),
        "B1": np.asarray(b1, f).reshape(H, 1),
        "LT_t1": np.ascontiguousarray(W1G[:H]),
        "LT_b1": np.ascontiguousarray(W1G[H:]),
        "onsq4": onsq.reshape(H, H),
        "E4": E4.reshape(H, ngrp * H),
        "LT_w2c": np.tile(np.asarray(W2, f).T, (1, 4)),
        "B2r": np.tile(np.asarray(b2, f), 4).reshape(H, 1),
    }
    for k, tname, bname in ((4, "LT_t4", "LT_b4"), (3, "LT_t3", "LT_b3"), (2, "LT_t2", "LT_b2")):
        scaled = np.tile(Gflat * f(1.0 / k), (1, 4))
        p[tname] = np.ascontiguousarray(scaled[:H])
        p[bname] = np.ascontiguousarray(scaled[H:])
    for name, (shape, dt) in _PARAM_SHAPES.items():
        assert list(p[name].shape) == shape, (name, p[name].shape, shape)
        if dt == BF16:
            p[name] = p[name].astype(bf)
        else:
            p[name] = np.ascontiguousarray(p[name], f)
    return p


def _run(z, G, W_se1, b_se1, W_se2, b_se2, W1, b1, W2, b2, trace=False, **trace_kw):
    import ml_dtypes
    z = np.asarray(z, np.float32)
    nt = BC // BT
    # b_se1/b1 go through ACT bias slots either way; only b_se2/b2 change
    # the instruction count.
    zero_bias = (float(np.abs(np.asarray(b_se2)).max()) == 0.0
                 and float(np.abs(np.asarray(b2)).max()) == 0.0)
    params = _host_params(G, W_se1, b_se1, W_se2, b_se2, W1, b1, W2, b2, nt)

    zT = np.ascontiguousarray(
        z.reshape(NCORES, BC, D).transpose(0, 2, 1)
    ).astype(ml_dtypes.bfloat16)

    nc = _build_program(BC, zero_bias)
    in_maps = [{"zT": zT[c], **params} for c in range(NCORES)]
    res = run_bass_kernel_spmd(nc, in_maps, list(range(NCORES)), trace=trace, **trace_kw)

    # out4 per core: [128, ngrp*BT]; row 32r+j, col g*BT+b -> sample (4g+r)*BT+b, feature j
    ngrp = (BC // BT) // GS
    out4 = np.stack([res.results[c]["out4"] for c in range(NCORES)])
    out = (out4.reshape(NCORES, GS, D, ngrp, BT)
           .transpose(0, 3, 1, 4, 2)          # core, g, r, b, j
           .reshape(B, D))
    return np.ascontiguousarray(out.astype(np.float32)), res


def kernel(z, G, W_se1, b_se1, W_se2, b_se2, W1, b1, W2, b2):
    out, _ = _run(z, G, W_se1, b_se1, W_se2, b_se2, W1, b1, W2, b2, trace=False)
    return out


if __name__ == "__main__":
    rng = np.random.default_rng(0)
    inputs = {
        "z": rng.standard_normal((B, D), dtype=np.float32),
        "G": (rng.standard_normal((NG, D, D)) * 0.1).astype(np.float32),
        "W_se1": (rng.standard_normal((H, D)) / np.sqrt(D)).astype(np.float32),
        "b_se1": np.zeros(H, np.float32),
        "W_se2": (rng.standard_normal((NG, H)) / np.sqrt(H)).astype(np.float32),
        "b_se2": np.zeros(NG, np.float32),
        "W1": (rng.standard_normal((H, D)) * 0.01).astype(np.float32),
        "b1": np.zeros(H, np.float32),
        "W2": (rng.standard_normal((D, H)) * 0.01).astype(np.float32),
        "b2": np.zeros(D, np.float32),
    }
    out = kernel(**inputs)
    print("kernel output", out.shape, out.dtype, float(np.abs(out).max()))


# revision 60
# speedup vs baseline: 1.0199x; 1.0199x over previous
"""EquivariantEvolution kernel for 8 Trainium2 NeuronCores (Bass/Tile).

Math (per sample):
    alpha = W_se2 silu(W_se1 z + b_se1) + b_se2            # [NG=8]
    A     = sum_g alpha_g G_g                              # [32, 32]
    z_t   = (I + A + A^2/2 + A^3/6 + A^4/24) z             # order-4 Taylor
    h1    = W1 z_t + b1                                    # [128]
    out   = sigmoid(|h1| + eps) * (W2 h1) + b2             # gate commuted past W2

Device strategy (pure batch data-parallel, feature-major [feat, samples]):
  * Horner: v <- z + (1/k) A v.  A v as y[(g,i),b] = alpha_g[b] v_i[b]
    (elementwise outer product) contracted by two K=128 matmuls whose
    lhsT is pre-replicated 4x along M so outputs land ready for the next
    elementwise step.  The +z fold is a K=32 matmul row-packed 2x via
    tile_position so pairs run concurrently in the PE array.
  * Everything the PE touches is bf16 (FWL weight loads, half DMA);
    all accumulation stays f32 in PSUM.
  * The norm-squared reduction for all 16 tiles accumulates into ONE
    PSUM bank (4 col-groups x 4 rows), so the sqrt/tanh run once per
    kernel (2 ACT table switches total).
  * Phase B: gate = 0.5*tanh+0.5 is broadcast to 4 tiles at once by a
    single 0/0.5 matmul; W2 h1 is col-packed 4x (M=32 each); one DVE
    multiply + one 4-strip DMA store finishes 4 tiles.
  * No HAM warm-up spam: a short burst of zero matmuls during the
    initial parameter DMAs brings the clock to K=8/8; after that the
    real matmul stream is dense enough to keep it there.
"""

import os
import sys

import numpy as np

for _p in ("/opt/trn_rl_repo", "/root/.axon_site/_ro/trn_rl_repo"):
    if os.path.isdir(_p) and _p not in sys.path:
        sys.path.insert(0, _p)

import concourse.bass as bass
import concourse.mybir as mybir
import concourse.tile as tile
from concourse.bass_utils import run_bass_kernel_spmd

B, D, H, NG = 65536, 32, 128, 8
NCORES = 8
BC = B // NCORES          # samples per core
BT = 512                  # samples per tile (PSUM bank width in f32)
GS = 4                    # tiles per group
EPS = 1e-6
F32 = mybir.dt.float32
F32R = mybir.dt.float32r
BF16 = mybir.dt.bfloat16
AF = mybir.ActivationFunctionType

# whether the z-path outer product (all-SBUF operands) runs on gpsimd
YCAT0_ON_GPSIMD = True


def _split_multi_waits(nc, max_waits=1):
    """This toolchain's walrus rejects >1 sync-wait on an instruction
    ("Too many sync wait commands"); hoist extra waits onto preceding
    same-engine NOPs (in-order engines make this semantics-preserving)."""
    n_new = 0
    for f in nc.m.functions:
        for bb in f.blocks:
            out = []
            for ins in bb.instructions:
                si = getattr(ins, "sync_info", None)
                if si is not None and si.on_wait and len(si.on_wait) > max_waits:
                    waits = list(si.on_wait)
                    chunks = [waits[i:i + max_waits] for i in range(0, len(waits), max_waits)]
                    for ci, ch in enumerate(chunks[:-1]):
                        nop = mybir.InstNoOp(
                            name=f"{ins.name}-wsplit{ci}",
                            engine=ins.engine,
                            sync_info=mybir.SyncInfo(on_wait=ch, on_update=[]),
                            bass_nofuse=True,
                        )
                        out.append(nop)
                        n_new += 1
                    ins.sync_info = mybir.SyncInfo(on_wait=chunks[-1], on_update=si.on_update)
                out.append(ins)
            bb.instructions[:] = out
    return n_new


# DRAM parameters: name -> (shape, dtype).  All matmul operands bf16.
_PARAM_SHAPES = {
    "LT_h4": ([H, H], BF16),      # W_se1^T tiled 4x along partitions
    "LT_At": ([H, H], BF16),      # W_se2[0:4] replicated 32x over M
    "LT_Ab": ([H, H], BF16),      # W_se2[4:8]
    "Bse1": ([H, 1], F32),
    "Bse2t": ([H, 1], F32),
    "Bse2b": ([H, 1], F32),
    "LT_t4": ([H, H], BF16), "LT_b4": ([H, H], BF16),
    "LT_t3": ([H, H], BF16), "LT_b3": ([H, H], BF16),
    "LT_t2": ([H, H], BF16), "LT_b2": ([H, H], BF16),
    "LT_t1": ([H, H], BF16), "LT_b1": ([H, H], BF16),   # W1-folded k=1 step
    "LT_z4": ([H, H], BF16),      # I32 tiled (4,4): row-packable +z fold
    "LT_w1z4": ([H, H], BF16),    # W1^T tiled 4x along partitions
    "B1": ([H, 1], F32),
    "onsq4": ([H, H], BF16),      # norm-sq row-select weights (variant r at cols 32r)
    "E4": ([H, 4 * H], BF16),     # 0.5-scaled gate broadcast, one [H,H] block per group
    "LT_w2c": ([H, H], BF16),     # W2^T tiled 4x along M (col-packable)
    "B2r": ([H, 1], F32),         # b2 tiled 4x along partitions
}


def _build_program(bc: int, zero_bias: bool, sim_safe: bool = False, split_waits: bool = True):
    nt = bc // BT
    ngrp = nt // GS
    nc = bass.Bass()

    zT = nc.declare_dram_parameter("zT", [D, bc], BF16, isOutput=False)
    params = {
        name: nc.declare_dram_parameter(name, shape, dt, isOutput=False)
        for name, (shape, dt) in _PARAM_SHAPES.items()
    }
    ngrp_ = (bc // BT) // GS
    out4 = nc.declare_dram_parameter("out4", [H, ngrp_ * BT], F32, isOutput=True)

    with tile.TileContext(nc) as tc:
        with (
            tc.tile_pool(name="consts", bufs=1) as consts,
            tc.tile_pool(name="zv4", bufs=3) as zv4_pool,
            tc.tile_pool(name="hs", bufs=3) as hs_pool,
            tc.tile_pool(name="acat", bufs=7) as acat_pool,   # [H,2,2,BT] pair tiles
            tc.tile_pool(name="ycat0", bufs=6) as ycat0_pool,  # long-lived z-products
            tc.tile_pool(name="ycat", bufs=5) as ycat_pool,   # short-lived step products
            tc.tile_pool(name="sq", bufs=5) as sq_pool,
            tc.tile_pool(name="h1s", bufs=nt) as h1s_pool,
            tc.tile_pool(name="gate", bufs=6) as gate_pool,
            tc.tile_pool(name="og", bufs=3) as og_pool,
            # PSUM: hp(1) + ap(1x2) + pv(2x2) + nsq(1) = 8 banks
            tc.tile_pool(name="hp", bufs=1, space=bass.MemorySpace.PSUM) as hp_pool,
            tc.tile_pool(name="ap", bufs=1, space=bass.MemorySpace.PSUM) as ap_pool,
            tc.tile_pool(name="pv", bufs=2, space=bass.MemorySpace.PSUM) as pv_pool,
            tc.tile_pool(name="psn", bufs=1, space=bass.MemorySpace.PSUM) as psn_pool,
        ):
            # ---- constants into SBUF (sync queue only: the scalar queue
            # would block the ACT FIFO, gpsimd carries the z loads) ----
            ct = {}
            qs = (nc.sync, nc.scalar, nc.sync, nc.gpsimd)
            for name, (shape, dt) in _PARAM_SHAPES.items():
                t = consts.tile(shape, dt, name=f"c_{name}")
                nc.sync.dma_start(t[:], params[name][:])
                ct[name] = t
            half_b = consts.tile([H, 1], F32, name="half_b")
            nc.vector.memset(half_b[:], 0.5)
            zero_b = consts.tile([H, 1], F32, name="zero_b")
            nc.vector.memset(zero_b[:], 0.0)
            tanh_b = consts.tile([H, 1], F32, name="tanh_b")
            nc.vector.memset(tanh_b[:], 0.5 * EPS)

            # ---- HAM warm filler: zero-matmuls accumulated into the live
            # norm-sq bank (wscr is all zeros, so they add exactly 0).  Keeps
            # the PE clock gate at K=8/8 wherever the real stream thins,
            # without needing a PSUM bank of their own. ----
            wscr = consts.tile([H, BT], BF16, name="wscr")
            nc.vector.memset(wscr[:], 0.0)
            nsq_cur = [psn_pool.tile([H, BT], F32, name="nsq_ps", tag="nsq")]

            def warm(n, cols=BT):
                for _ in range(n):
                    nc.tensor.matmul(nsq_cur[0][:, 0:cols], wscr[:, 0:H],
                                     wscr[:, 0:cols],
                                     start=False, stop=False, skip_group_check=True)

            warm(20)

            taylor = [
                (ct["LT_t4"], ct["LT_b4"]),
                (ct["LT_t3"], ct["LT_b3"]),
                (ct["LT_t2"], ct["LT_b2"]),
            ]

            h1s_tiles = []
            PAIRS = (0, 2)

            def emit_loads(g):
                zv4 = zv4_pool.tile([H, GS, BT], BF16, name="zv4")
                for s in range(4):
                    nc.gpsimd.dma_start(
                        zv4[32 * s:32 * (s + 1), :, :],
                        zT[:, bass.ts(g, GS * BT)],
                    )
                return zv4

            def emit_extractor(zv4, g=99):
                """alpha for 4 tiles; returns per-pair acat/ycat0 [H,2,2,BT]."""
                acats, ycats = {}, {}
                for jp in PAIRS:
                    acat = acat_pool.tile([H, 2, 2, BT], BF16, name="acat")
                    for jj in range(2):
                        j = jp + jj
                        hp = hp_pool.tile([H, BT], F32, name="hp", tag="hp")
                        nc.tensor.matmul(hp[:], ct["LT_h4"][0:D, :], zv4[0:D, j, :],
                                         start=True, stop=True)
                        hs = hs_pool.tile([H, BT], BF16, name="hs")
                        if sim_safe:
                            sg = hs_pool.tile([H, BT], F32, name="sg")
                            nc.scalar.activation(sg[:], hp[:], AF.Sigmoid, bias=ct["Bse1"][:])
                            hx = hs_pool.tile([H, BT], F32, name="hx")
                            nc.scalar.activation(hx[:], hp[:], AF.Identity, bias=ct["Bse1"][:])
                            nc.vector.tensor_tensor(hs[:], sg[:], hx[:], mybir.AluOpType.mult)
                        else:
                            nc.scalar.activation(hs[:], hp[:], AF.Silu, bias=ct["Bse1"][:])

                        ap = ap_pool.tile([H, 2, BT], F32, name="ap", tag="ap")
                        nc.tensor.matmul(ap[:, 0, :], ct["LT_At"][:], hs[:],
                                         start=True, stop=True)
                        nc.tensor.matmul(ap[:, 1, :], ct["LT_Ab"][:], hs[:],
                                         start=True, stop=True)
                        if zero_bias:
                            nc.scalar.activation(acat[:, :, jj, :], ap[:], AF.Identity)
                        else:
                            nc.scalar.activation(acat[:, 0, jj, :], ap[:, 0, :], AF.Identity,
                                                 bias=ct["Bse2t"][:])
                            nc.scalar.activation(acat[:, 1, jj, :], ap[:, 1, :], AF.Identity,
                                                 bias=ct["Bse2b"][:])
                    acats[jp] = acat

                    ycat = ycat0_pool.tile([H, 2, 2, BT], BF16, name="ycat0")
                    # ramp: DVE is idle for the first two groups; steady state
                    # keeps the z-path product off the loaded DVE
                    eng = nc.gpsimd if (YCAT0_ON_GPSIMD and g >= 2) else nc.vector
                    eng.tensor_tensor(
                        ycat[:], acat[:],
                        zv4[:, None, jp:jp + 2, :].broadcast_to([H, 2, 2, BT]),
                        mybir.AluOpType.mult,
                    )
                    ycats[jp] = ycat
                return acats, ycats

            def emit_taylor(zv4, acats, ycats):
                """Horner k=4,3,2 then W1-fused k=1; h1s/sq per tile."""
                for lt_top, lt_bot in taylor:
                    for jp in PAIRS:
                        pv2 = pv_pool.tile([H, 2, BT], F32, name="pv2", tag="pv")
                        for jj in range(2):
                            j = jp + jj
                            nc.tensor.matmul(
                                pv2[:, jj, :], ct["LT_z4"][bass.ts(j, 32), :],
                                zv4[bass.ts(j, 32), j, :],
                                start=True, stop=False, skip_group_check=True,
                                tile_position=(32 * j, 0),
                            )
                        for jj in range(2):  # tops then bots: adjacent same-weight MMs
                            nc.tensor.matmul(pv2[:, jj, :], lt_top[:],
                                             ycats[jp][:, 0, jj, :],
                                             start=False, stop=False, skip_group_check=True)
                        for jj in range(2):
                            nc.tensor.matmul(pv2[:, jj, :], lt_bot[:],
                                             ycats[jp][:, 1, jj, :],
                                             start=False, stop=True, skip_group_check=True)
                        ycat = ycat_pool.tile([H, 2, 2, BT], BF16, name="ycat")
                        nc.vector.tensor_tensor(
                            ycat[:], acats[jp][:],
                            pv2[:, None, :, :].broadcast_to([H, 2, 2, BT]),
                            mybir.AluOpType.mult,
                        )
                        ycats[jp] = ycat
                        warm(1)

                sqs = {}
                for jp in PAIRS:
                    h1p2 = pv_pool.tile([H, 2, BT], F32, name="h1p2", tag="pv")
                    for jj in range(2):
                        j = jp + jj
                        nc.tensor.matmul(
                            h1p2[:, jj, :], ct["LT_w1z4"][bass.ts(j, 32), :],
                            zv4[bass.ts(j, 32), j, :],
                            start=True, stop=False, skip_group_check=True,
                            tile_position=(32 * j, 0),
                        )
                    for jj in range(2):
                        nc.tensor.matmul(h1p2[:, jj, :], ct["LT_t1"][:],
                                         ycats[jp][:, 0, jj, :],
                                         start=False, stop=False, skip_group_check=True)
                    for jj in range(2):
                        nc.tensor.matmul(h1p2[:, jj, :], ct["LT_b1"][:],
                                         ycats[jp][:, 1, jj, :],
                                         start=False, stop=True, skip_group_check=True)
                    for jj in range(2):  # sq first: it feeds the gate critical path
                        j = jp + jj
                        sq = sq_pool.tile([H, BT], BF16, name="sq")
                        nc.scalar.activation(sq[:], h1p2[:, jj, :], AF.Square,
                                             bias=ct["B1"][:])
                        sqs[j] = sq
                    for jj in range(2):
                        h1s = h1s_pool.tile([H, BT], BF16, name="h1s")
                        nc.scalar.activation(h1s[:], h1p2[:, jj, :], AF.Identity,
                                             bias=ct["B1"][:])
                        h1s_tiles.append(h1s)
                    warm(1)
                return sqs

            def emit_nsq(g, sqs, nsq_ps):
                # norm-sq of tile 4g+j -> col-group j, row g%2 (4 col-packed MMs);
                # each nsq bank covers two groups.
                for j in range(GS):
                    nc.tensor.matmul(
                        nsq_ps[bass.ts(j, 32), :], ct["onsq4"][:, bass.ts(g % 2, 32)],
                        sqs[j][:],
                        start=(g % 2 == 0), stop=(g % 2 == 1), skip_group_check=True,
                        tile_position=(0, 32 * j),
                    )

            def emit_gate(nsq_ps):
                """batched sqrt + tanh for the two groups in one nsq bank"""
                rt = gate_pool.tile([H, BT], F32, name="rt")
                nc.scalar.activation(rt[:], nsq_ps[:], AF.Sqrt, bias=zero_b[:])
                t_all = gate_pool.tile([H, BT], BF16, name="t_all")
                # sigmoid(norm + eps) = 0.5 tanh(0.5 norm + eps/2) + 0.5
                nc.scalar.activation(t_all[:], rt[:], AF.Tanh, scale=0.5, bias=tanh_b[:])
                return t_all

            def emit_w2(gpair, pool):
                """gate-independent W2 h1 for two groups into one pair tile"""
                wpp = pool.tile([H, 2, BT], F32, name="wp2",
                                tag="ap" if pool is ap_pool else "pv")
                for gi in range(2):
                    for j in range(GS):  # col-packed W2 h1, M=32 each
                        nc.tensor.matmul(
                            wpp[bass.ts(j, 32), gi, :],
                            ct["LT_w2c"][:, bass.ts(j, 32)],
                            h1s_tiles[GS * (gpair + gi) + j][:],
                            start=True, stop=True, skip_group_check=True,
                            tile_position=(0, 32 * j),
                        )
                return wpp

            def emit_gate_b(g, t_all, wp):
                """gate-dependent part: broadcast, multiply, store"""
                trp = hp_pool.tile([H, BT], F32, name="trp", tag="hp")
                nc.tensor.matmul(trp[:], ct["E4"][:, bass.ts(g, H)], t_all[:],
                                 start=True, stop=True, skip_group_check=True)
                t2g = gate_pool.tile([H, BT], BF16, name="t2g")
                nc.scalar.activation(t2g[:], trp[:], AF.Identity, bias=half_b[:])
                og = og_pool.tile([H, BT], F32, name="og")
                nc.vector.tensor_tensor(og[:], wp, t2g[:], mybir.AluOpType.mult)
                if not zero_bias:
                    og2 = og_pool.tile([H, BT], F32, name="og2")
                    nc.scalar.activation(og2[:], og[:], AF.Identity, bias=ct["B2r"][:])
                    og = og2
                qs[g % 2].dma_start(out4[:, bass.ts(g, BT)], og[:])

            # ===== main schedule: pipelined phase A, gate split in halves so
            # phase B of groups 0-1 overlaps phase A of groups 2-3 =====
            from collections import deque
            pipe = deque()
            for g in range(min(2, ngrp)):
                zv4_g = emit_loads(g)
                pipe.append((zv4_g,) + emit_extractor(zv4_g, g))
            nsq_ps = None
            t_allA = None
            for g in range(ngrp):
                zv4_g, acats_g, ycats_g = pipe.popleft()
                if g + 2 < ngrp:
                    zv4_n = emit_loads(g + 2)
                    pipe.append((zv4_n,) + emit_extractor(zv4_n, g + 2))
                sqs = emit_taylor(zv4_g, acats_g, ycats_g)
                if g == 2:
                    nsq_cur[0] = psn_pool.tile([H, BT], F32, name="nsq_ps", tag="nsq")
                nsq_ps = nsq_cur[0]
                emit_nsq(g, sqs, nsq_ps)
                if g == 1:
                    t_allA = emit_gate(nsq_ps)
                elif g == 2:
                    wpA = emit_w2(0, ap_pool)  # ap pool is free after the last extractor
                    emit_gate_b(0, t_allA, wpA[:, 0, :])
                elif g == 3:
                    emit_gate_b(1, t_allA, wpA[:, 1, :])
                    wpB = emit_w2(2, pv_pool)  # pv pool free once phase A drains
                    t_allB = emit_gate(nsq_ps)
                    emit_gate_b(2, t_allB, wpB[:, 0, :])
                    emit_gate_b(3, t_allB, wpB[:, 1, :])

    if split_waits:
        _split_multi_waits(nc)
    return nc


def _host_params(G, W_se1, b_se1, W_se2, b_se2, W1, b1, W2, b2, nt):
    import ml_dtypes
    f = np.float32
    bf = ml_dtypes.bfloat16
    ngrp = nt // GS
    G = np.asarray(G, f)
    Gflat = np.transpose(G, (0, 2, 1)).reshape(NG * D, D)  # [(g,i), j] = G[g,j,i]
    W1G = Gflat @ np.asarray(W1, f).T                      # [(g,i), m]
    I32 = np.eye(D, dtype=f)

    onsq = np.zeros((H, 4, 32), f)
    for r in range(4):
        onsq[:, r, r] = 1.0
    E4 = np.zeros((H, ngrp, H), f)
    for g in range(ngrp):
        for r in range(GS):
            E4[32 * r + (g % 2), g, 32 * r:32 * (r + 1)] = 0.5

    p = {
        "LT_h4": np.tile(np.asarray(W_se1, f).T, (4, 1)),
        "LT_At": np.repeat(np.asarray(W_se2, f).T[:, 0:4], 32, axis=1),
        "LT_Ab": np.repeat(np.asarray(W_se2, f).T[:, 4:8], 32, axis=1),
        "Bse1": np.asarray(b_se1, f).reshape(H, 1),
        "Bse2t": np.repeat(np.asarray(b_se2, f)[0:4], 32).reshape(H, 1),
        "Bse2b": np.repeat(np.asarray(b_se2, f)[4:8], 32).reshape(H, 1),
        "LT_z4": np.tile(I32, (4, 4)),
        "LT_w1z4": np.tile(np.asarray(W1, f).T, (4, 1)),
        "B1": np.asarray(b1, f).reshape(H, 1),
        "LT_t1": np.ascontiguousarray(W1G[:H]),
        "LT_b1": np.ascontiguousarray(W1G[H:]),
        "onsq4": onsq.reshape(H, H),
        "E4": E4.reshape(H, ngrp * H),
        "LT_w2c": np.tile(np.asarray(W2, f).T, (1, 4)),
        "B2r": np.tile(np.asarray(b2, f), 4).reshape(H, 1),
    }
    for k, tname, bname in ((4, "LT_t4", "LT_b4"), (3, "LT_t3", "LT_b3"), (2, "LT_t2", "LT_b2")):
        scaled = np.tile(Gflat * f(1.0 / k), (1, 4))
        p[tname] = np.ascontiguousarray(scaled[:H])
        p[bname] = np.ascontiguousarray(scaled[H:])
    for name, (shape, dt) in _PARAM_SHAPES.items():
        assert list(p[name].shape) == shape, (name, p[name].shape, shape)
        if dt == BF16:
            p[name] = p[name].astype(bf)
        else:
            p[name] = np.ascontiguousarray(p[name], f)
    return p


def _run(z, G, W_se1, b_se1, W_se2, b_se2, W1, b1, W2, b2, trace=False, **trace_kw):
    import ml_dtypes
    z = np.asarray(z, np.float32)
    nt = BC // BT
    # b_se1/b1 go through ACT bias slots either way; only b_se2/b2 change
    # the instruction count.
    zero_bias = (float(np.abs(np.asarray(b_se2)).max()) == 0.0
                 and float(np.abs(np.asarray(b2)).max()) == 0.0)
    params = _host_params(G, W_se1, b_se1, W_se2, b_se2, W1, b1, W2, b2, nt)

    zT = np.ascontiguousarray(
        z.reshape(NCORES, BC, D).transpose(0, 2, 1)
    ).astype(ml_dtypes.bfloat16)

    nc = _build_program(BC, zero_bias)
    in_maps = [{"zT": zT[c], **params} for c in range(NCORES)]
    res = run_bass_kernel_spmd(nc, in_maps, list(range(NCORES)), trace=trace, **trace_kw)

    # out4 per core: [128, ngrp*BT]; row 32r+j, col g*BT+b -> sample (4g+r)*BT+b, feature j
    ngrp = (BC // BT) // GS
    out4 = np.stack([res.results[c]["out4"] for c in range(NCORES)])
    out = (out4.reshape(NCORES, GS, D, ngrp, BT)
           .transpose(0, 3, 1, 4, 2)          # core, g, r, b, j
           .reshape(B, D))
    return np.ascontiguousarray(out.astype(np.float32)), res


def kernel(z, G, W_se1, b_se1, W_se2, b_se2, W1, b1, W2, b2):
    out, _ = _run(z, G, W_se1, b_se1, W_se2, b_se2, W1, b1, W2, b2, trace=False)
    return out


if __name__ == "__main__":
    rng = np.random.default_rng(0)
    inputs = {
        "z": rng.standard_normal((B, D), dtype=np.float32),
        "G": (rng.standard_normal((NG, D, D)) * 0.1).astype(np.float32),
        "W_se1": (rng.standard_normal((H, D)) / np.sqrt(D)).astype(np.float32),
        "b_se1": np.zeros(H, np.float32),
        "W_se2": (rng.standard_normal((NG, H)) / np.sqrt(H)).astype(np.float32),
        "b_se2": np.zeros(NG, np.float32),
        "W1": (rng.standard_normal((H, D)) * 0.01).astype(np.float32),
        "b1": np.zeros(H, np.float32),
        "W2": (rng.standard_normal((D, H)) * 0.01).astype(np.float32),
        "b2": np.zeros(D, np.float32),
    }
    out = kernel(**inputs)
    print("kernel output", out.shape, out.dtype, float(np.abs(out).max()))


# revision 64
# speedup vs baseline: 1.0479x; 1.0275x over previous
"""EquivariantEvolution kernel for 8 Trainium2 NeuronCores (Bass/Tile).

Math (per sample):
    alpha = W_se2 silu(W_se1 z + b_se1) + b_se2            # [NG=8]
    A     = sum_g alpha_g G_g                              # [32, 32]
    z_t   = (I + A + A^2/2 + A^3/6 + A^4/24) z             # order-4 Taylor
    h1    = W1 z_t + b1                                    # [128]
    out   = sigmoid(|h1| + eps) * (W2 h1) + b2             # gate commuted past W2

Device strategy (pure batch data-parallel, feature-major [feat, samples]):
  * Horner: v <- z + (1/k) A v.  A v as y[(g,i),b] = alpha_g[b] v_i[b]
    (elementwise outer product) contracted by two K=128 matmuls whose
    lhsT is pre-replicated 4x along M so outputs land ready for the next
    elementwise step.  The +z fold is a K=32 matmul row-packed 2x via
    tile_position so pairs run concurrently in the PE array.
  * Everything the PE touches is bf16 (FWL weight loads, half DMA);
    all accumulation stays f32 in PSUM.
  * The norm-squared reduction for all 16 tiles accumulates into ONE
    PSUM bank (4 col-groups x 4 rows), so the sqrt/tanh run once per
    kernel (2 ACT table switches total).
  * Phase B: gate = 0.5*tanh+0.5 is broadcast to 4 tiles at once by a
    single 0/0.5 matmul; W2 h1 is col-packed 4x (M=32 each); one DVE
    multiply + one 4-strip DMA store finishes 4 tiles.
  * No HAM warm-up spam: a short burst of zero matmuls during the
    initial parameter DMAs brings the clock to K=8/8; after that the
    real matmul stream is dense enough to keep it there.
"""

import os
import sys

import numpy as np

for _p in ("/opt/trn_rl_repo", "/root/.axon_site/_ro/trn_rl_repo"):
    if os.path.isdir(_p) and _p not in sys.path:
        sys.path.insert(0, _p)

import concourse.bass as bass
import concourse.mybir as mybir
import concourse.tile as tile
from concourse.bass_utils import run_bass_kernel_spmd

B, D, H, NG = 65536, 32, 128, 8
NCORES = 8
BC = B // NCORES          # samples per core
BT = 512                  # samples per tile (PSUM bank width in f32)
GS = 4                    # tiles per group
EPS = 1e-6
F32 = mybir.dt.float32
F32R = mybir.dt.float32r
BF16 = mybir.dt.bfloat16
AF = mybir.ActivationFunctionType

# whether the z-path outer product (all-SBUF operands) runs on gpsimd
YCAT0_ON_GPSIMD = True


def _split_multi_waits(nc, max_waits=1):
    """This toolchain's walrus rejects >1 sync-wait on an instruction
    ("Too many sync wait commands"); hoist extra waits onto preceding
    same-engine NOPs (in-order engines make this semantics-preserving)."""
    n_new = 0
    for f in nc.m.functions:
        for bb in f.blocks:
            out = []
            for ins in bb.instructions:
                si = getattr(ins, "sync_info", None)
                if si is not None and si.on_wait and len(si.on_wait) > max_waits:
                    waits = list(si.on_wait)
                    chunks = [waits[i:i + max_waits] for i in range(0, len(waits), max_waits)]
                    for ci, ch in enumerate(chunks[:-1]):
                        nop = mybir.InstNoOp(
                            name=f"{ins.name}-wsplit{ci}",
                            engine=ins.engine,
                            sync_info=mybir.SyncInfo(on_wait=ch, on_update=[]),
                            bass_nofuse=True,
                        )
                        out.append(nop)
                        n_new += 1
                    ins.sync_info = mybir.SyncInfo(on_wait=chunks[-1], on_update=si.on_update)
                out.append(ins)
            bb.instructions[:] = out
    return n_new


# DRAM parameters: name -> (shape, dtype).  All matmul operands bf16.
_PARAM_SHAPES = {
    "LT_h4": ([H, H], BF16),      # W_se1^T tiled 4x along partitions
    "LT_At": ([H, H], BF16),      # W_se2[0:4] replicated 32x over M
    "LT_Ab": ([H, H], BF16),      # W_se2[4:8]
    "Bse1": ([H, 1], F32),
    "Bse2t": ([H, 1], F32),
    "Bse2b": ([H, 1], F32),
    "LT_t4": ([H, H], BF16), "LT_b4": ([H, H], BF16),
    "LT_t3": ([H, H], BF16), "LT_b3": ([H, H], BF16),
    "LT_t2": ([H, H], BF16), "LT_b2": ([H, H], BF16),
    "LT_t1": ([H, H], BF16), "LT_b1": ([H, H], BF16),   # W1-folded k=1 step
    "LT_z4": ([H, H], BF16),      # I32 tiled (4,4): row-packable +z fold
    "LT_w1z4": ([H, H], BF16),    # W1^T tiled 4x along partitions
    "B1": ([H, 1], F32),
    "onsq4": ([H, H], BF16),      # norm-sq row-select weights (variant r at cols 32r)
    "E4": ([H, 4 * H], BF16),     # 0.5-scaled gate broadcast, one [H,H] block per group
    "LT_w2c": ([H, H], BF16),     # W2^T tiled 4x along M (col-packable)
    "B2r": ([H, 1], F32),         # b2 tiled 4x along partitions
}


def _build_program(bc: int, zero_bias: bool, sim_safe: bool = False, split_waits: bool = True):
    nt = bc // BT
    ngrp = nt // GS
    nc = bass.Bass()

    zT = nc.declare_dram_parameter("zT", [D, bc], BF16, isOutput=False)
    params = {
        name: nc.declare_dram_parameter(name, shape, dt, isOutput=False)
        for name, (shape, dt) in _PARAM_SHAPES.items()
    }
    ngrp_ = (bc // BT) // GS
    out4 = nc.declare_dram_parameter("out4", [H, ngrp_ * BT], F32, isOutput=True)

    with tile.TileContext(nc) as tc:
        with (
            tc.tile_pool(name="consts", bufs=1) as consts,
            tc.tile_pool(name="zv4", bufs=3) as zv4_pool,
            tc.tile_pool(name="hs", bufs=3) as hs_pool,
            tc.tile_pool(name="acat", bufs=7) as acat_pool,   # [H,2,2,BT] pair tiles
            tc.tile_pool(name="ycat0", bufs=6) as ycat0_pool,  # long-lived z-products
            tc.tile_pool(name="ycat", bufs=5) as ycat_pool,   # short-lived step products
            tc.tile_pool(name="sq", bufs=5) as sq_pool,
            tc.tile_pool(name="h1s", bufs=nt) as h1s_pool,
            tc.tile_pool(name="gate", bufs=6) as gate_pool,
            tc.tile_pool(name="og", bufs=3) as og_pool,
            # PSUM: hp(1) + ap(1x2) + pv(2x2) + nsq(1) = 8 banks
            tc.tile_pool(name="hp", bufs=1, space=bass.MemorySpace.PSUM) as hp_pool,
            tc.tile_pool(name="ap", bufs=1, space=bass.MemorySpace.PSUM) as ap_pool,
            tc.tile_pool(name="pv", bufs=2, space=bass.MemorySpace.PSUM) as pv_pool,
            tc.tile_pool(name="psn", bufs=1, space=bass.MemorySpace.PSUM) as psn_pool,
        ):
            # ---- constants into SBUF (sync queue only: the scalar queue
            # would block the ACT FIFO, gpsimd carries the z loads) ----
            ct = {}
            qs = (nc.sync, nc.scalar, nc.sync, nc.gpsimd)
            for name, (shape, dt) in _PARAM_SHAPES.items():
                t = consts.tile(shape, dt, name=f"c_{name}")
                nc.sync.dma_start(t[:], params[name][:])
                ct[name] = t
            half_b = consts.tile([H, 1], F32, name="half_b")
            nc.vector.memset(half_b[:], 0.5)
            zero_b = consts.tile([H, 1], F32, name="zero_b")
            nc.vector.memset(zero_b[:], 0.0)
            tanh_b = consts.tile([H, 1], F32, name="tanh_b")
            nc.vector.memset(tanh_b[:], 0.5 * EPS)

            # ---- HAM warm filler: zero-matmuls accumulated into the live
            # norm-sq bank (wscr is all zeros, so they add exactly 0).  Keeps
            # the PE clock gate at K=8/8 wherever the real stream thins,
            # without needing a PSUM bank of their own. ----
            wscr = consts.tile([H, BT], BF16, name="wscr")
            nc.vector.memset(wscr[:], 0.0)
            nsq_cur = [psn_pool.tile([H, BT], F32, name="nsq_ps", tag="nsq")]

            def warm(n, cols=BT):
                for _ in range(n):
                    nc.tensor.matmul(nsq_cur[0][:, 0:cols], wscr[:, 0:H],
                                     wscr[:, 0:cols],
                                     start=False, stop=False, skip_group_check=True)

            warm(20)

            taylor = [
                (ct["LT_t4"], ct["LT_b4"]),
                (ct["LT_t3"], ct["LT_b3"]),
                (ct["LT_t2"], ct["LT_b2"]),
            ]

            h1s_tiles = []
            PAIRS = (0, 2)

            def emit_loads(g):
                zv4 = zv4_pool.tile([H, GS, BT], BF16, name="zv4")
                for s in range(4):
                    nc.gpsimd.dma_start(
                        zv4[32 * s:32 * (s + 1), :, :],
                        zT[:, bass.ts(g, GS * BT)],
                    )
                return zv4

            def emit_extractor(zv4, g=99):
                """alpha for 4 tiles; returns per-pair acat/ycat0 [H,2,2,BT]."""
                acats, ycats = {}, {}
                for jp in PAIRS:
                    acat = acat_pool.tile([H, 2, 2, BT], BF16, name="acat")
                    for jj in range(2):
                        j = jp + jj
                        hp = hp_pool.tile([H, BT], F32, name="hp", tag="hp")
                        nc.tensor.matmul(hp[:], ct["LT_h4"][0:D, :], zv4[0:D, j, :],
                                         start=True, stop=True)
                        hs = hs_pool.tile([H, BT], BF16, name="hs")
                        if sim_safe:
                            sg = hs_pool.tile([H, BT], F32, name="sg")
                            nc.scalar.activation(sg[:], hp[:], AF.Sigmoid, bias=ct["Bse1"][:])
                            hx = hs_pool.tile([H, BT], F32, name="hx")
                            nc.scalar.activation(hx[:], hp[:], AF.Identity, bias=ct["Bse1"][:])
                            nc.vector.tensor_tensor(hs[:], sg[:], hx[:], mybir.AluOpType.mult)
                        else:
                            nc.scalar.activation(hs[:], hp[:], AF.Silu, bias=ct["Bse1"][:])

                        ap = ap_pool.tile([H, 2, BT], F32, name="ap", tag="ap")
                        nc.tensor.matmul(ap[:, 0, :], ct["LT_At"][:], hs[:],
                                         start=True, stop=True)
                        nc.tensor.matmul(ap[:, 1, :], ct["LT_Ab"][:], hs[:],
                                         start=True, stop=True)
                        if zero_bias:
                            nc.scalar.activation(acat[:, :, jj, :], ap[:], AF.Identity)
                        else:
                            nc.scalar.activation(acat[:, 0, jj, :], ap[:, 0, :], AF.Identity,
                                                 bias=ct["Bse2t"][:])
                            nc.scalar.activation(acat[:, 1, jj, :], ap[:, 1, :], AF.Identity,
                                                 bias=ct["Bse2b"][:])
                    acats[jp] = acat

                    ycat = ycat0_pool.tile([H, 2, 2, BT], BF16, name="ycat0")
                    # ramp: DVE is idle for the first two groups; steady state
                    # keeps the z-path product off the loaded DVE
                    eng = nc.gpsimd if (YCAT0_ON_GPSIMD and g >= 2) else nc.vector
                    eng.tensor_tensor(
                        ycat[:], acat[:],
                        zv4[:, None, jp:jp + 2, :].broadcast_to([H, 2, 2, BT]),
                        mybir.AluOpType.mult,
                    )
                    ycats[jp] = ycat
                return acats, ycats

            def emit_taylor(zv4, acats, ycats, defer_h1s=False):
                """Horner k=4,3,2 then W1-fused k=1; h1s/sq per tile."""
                for lt_top, lt_bot in taylor:
                    for jp in PAIRS:
                        pv2 = pv_pool.tile([H, 2, BT], F32, name="pv2", tag="pv")
                        for jj in range(2):
                            j = jp + jj
                            nc.tensor.matmul(
                                pv2[:, jj, :], ct["LT_z4"][bass.ts(j, 32), :],
                                zv4[bass.ts(j, 32), j, :],
                                start=True, stop=False, skip_group_check=True,
                                tile_position=(32 * j, 0),
                            )
                        for jj in range(2):  # tops then bots: adjacent same-weight MMs
                            nc.tensor.matmul(pv2[:, jj, :], lt_top[:],
                                             ycats[jp][:, 0, jj, :],
                                             start=False, stop=False, skip_group_check=True)
                        for jj in range(2):
                            nc.tensor.matmul(pv2[:, jj, :], lt_bot[:],
                                             ycats[jp][:, 1, jj, :],
                                             start=False, stop=True, skip_group_check=True)
                        ycat = ycat_pool.tile([H, 2, 2, BT], BF16, name="ycat")
                        nc.vector.tensor_tensor(
                            ycat[:], acats[jp][:],
                            pv2[:, None, :, :].broadcast_to([H, 2, 2, BT]),
                            mybir.AluOpType.mult,
                        )
                        ycats[jp] = ycat
                        warm(1)

                sqs = {}
                h1s_deferred = []
                for jp in PAIRS:
                    h1p2 = pv_pool.tile([H, 2, BT], F32, name="h1p2", tag="pv")
                    for jj in range(2):
                        j = jp + jj
                        nc.tensor.matmul(
                            h1p2[:, jj, :], ct["LT_w1z4"][bass.ts(j, 32), :],
                            zv4[bass.ts(j, 32), j, :],
                            start=True, stop=False, skip_group_check=True,
                            tile_position=(32 * j, 0),
                        )
                    for jj in range(2):
                        nc.tensor.matmul(h1p2[:, jj, :], ct["LT_t1"][:],
                                         ycats[jp][:, 0, jj, :],
                                         start=False, stop=False, skip_group_check=True)
                    for jj in range(2):
                        nc.tensor.matmul(h1p2[:, jj, :], ct["LT_b1"][:],
                                         ycats[jp][:, 1, jj, :],
                                         start=False, stop=True, skip_group_check=True)
                    for jj in range(2):  # sq first: it feeds the gate critical path
                        j = jp + jj
                        sq = sq_pool.tile([H, BT], BF16, name="sq")
                        nc.scalar.activation(sq[:], h1p2[:, jj, :], AF.Square,
                                             bias=ct["B1"][:])
                        sqs[j] = sq
                    def _h1s(h1p2=h1p2):
                        for jj in range(2):
                            h1s = h1s_pool.tile([H, BT], BF16, name="h1s")
                            nc.scalar.activation(h1s[:], h1p2[:, jj, :], AF.Identity,
                                                 bias=ct["B1"][:])
                            h1s_tiles.append(h1s)
                    if defer_h1s:
                        h1s_deferred.append(_h1s)
                    else:
                        _h1s()
                    warm(1)
                return sqs, h1s_deferred

            def emit_nsq(g, sqs, nsq_ps):
                # norm-sq of tile 4g+j -> col-group j, row g%2 (4 col-packed MMs);
                # each nsq bank covers two groups.
                for j in range(GS):
                    nc.tensor.matmul(
                        nsq_ps[bass.ts(j, 32), :], ct["onsq4"][:, bass.ts(g % 2, 32)],
                        sqs[j][:],
                        start=(g % 2 == 0), stop=(g % 2 == 1), skip_group_check=True,
                        tile_position=(0, 32 * j),
                    )

            def emit_gate(nsq_ps):
                """batched sqrt + tanh for the two groups in one nsq bank"""
                rt = gate_pool.tile([H, BT], F32, name="rt")
                nc.scalar.activation(rt[:], nsq_ps[:], AF.Sqrt, bias=zero_b[:])
                t_all = gate_pool.tile([H, BT], BF16, name="t_all")
                # sigmoid(norm + eps) = 0.5 tanh(0.5 norm + eps/2) + 0.5
                nc.scalar.activation(t_all[:], rt[:], AF.Tanh, scale=0.5, bias=tanh_b[:])
                return t_all

            def emit_w2(gpair, pool):
                """gate-independent W2 h1 for two groups into one pair tile"""
                wpp = pool.tile([H, 2, BT], F32, name="wp2",
                                tag="ap" if pool is ap_pool else "pv")
                for gi in range(2):
                    for j in range(GS):  # col-packed W2 h1, M=32 each
                        nc.tensor.matmul(
                            wpp[bass.ts(j, 32), gi, :],
                            ct["LT_w2c"][:, bass.ts(j, 32)],
                            h1s_tiles[GS * (gpair + gi) + j][:],
                            start=True, stop=True, skip_group_check=True,
                            tile_position=(0, 32 * j),
                        )
                return wpp

            def emit_gate_b(g, t_all, wp):
                """gate-dependent part: broadcast, multiply, store"""
                trp = hp_pool.tile([H, BT], F32, name="trp", tag="hp")
                nc.tensor.matmul(trp[:], ct["E4"][:, bass.ts(g, H)], t_all[:],
                                 start=True, stop=True, skip_group_check=True)
                t2g = gate_pool.tile([H, BT], BF16, name="t2g")
                nc.scalar.activation(t2g[:], trp[:], AF.Identity, bias=half_b[:])
                og = og_pool.tile([H, BT], F32, name="og")
                nc.vector.tensor_tensor(og[:], wp, t2g[:], mybir.AluOpType.mult)
                if not zero_bias:
                    og2 = og_pool.tile([H, BT], F32, name="og2")
                    nc.scalar.activation(og2[:], og[:], AF.Identity, bias=ct["B2r"][:])
                    og = og2
                qs[g % 2].dma_start(out4[:, bass.ts(g, BT)], og[:])

            # ===== main schedule: pipelined phase A, gate split in halves so
            # phase B of groups 0-1 overlaps phase A of groups 2-3 =====
            from collections import deque
            pipe = deque()
            for g in range(min(2, ngrp)):
                zv4_g = emit_loads(g)
                pipe.append((zv4_g,) + emit_extractor(zv4_g, g))
            nsq_ps = None
            t_allA = None
            for g in range(ngrp):
                zv4_g, acats_g, ycats_g = pipe.popleft()
                if g + 2 < ngrp:
                    zv4_n = emit_loads(g + 2)
                    pipe.append((zv4_n,) + emit_extractor(zv4_n, g + 2))
                sqs, h1s_def = emit_taylor(zv4_g, acats_g, ycats_g,
                                           defer_h1s=(g == ngrp - 1))
                if g == 2:
                    nsq_cur[0] = psn_pool.tile([H, BT], F32, name="nsq_ps", tag="nsq")
                nsq_ps = nsq_cur[0]
                emit_nsq(g, sqs, nsq_ps)
                if g == 1:
                    t_allA = emit_gate(nsq_ps)
                elif g == 2:
                    wpA = emit_w2(0, ap_pool)  # ap pool is free after the last extractor
                    emit_gate_b(0, t_allA, wpA[:, 0, :])
                    emit_gate_b(1, t_allA, wpA[:, 1, :])
                elif g == 3:
                    # gate ACTs jump the queue; the deferred h1s follow them
                    t_allB = emit_gate(nsq_ps)
                    for fn in h1s_def:
                        fn()
                    wpB = emit_w2(2, pv_pool)  # pv pool free once phase A drains
                    emit_gate_b(2, t_allB, wpB[:, 0, :])
                    emit_gate_b(3, t_allB, wpB[:, 1, :])

    if split_waits:
        _split_multi_waits(nc)
    return nc


def _host_params(G, W_se1, b_se1, W_se2, b_se2, W1, b1, W2, b2, nt):
    import ml_dtypes
    f = np.float32
    bf = ml_dtypes.bfloat16
    ngrp = nt // GS
    G = np.asarray(G, f)
    Gflat = np.transpose(G, (0, 2, 1)).reshape(NG * D, D)  # [(g,i), j] = G[g,j,i]
    W1G = Gflat @ np.asarray(W1, f).T                      # [(g,i), m]
    I32 = np.eye(D, dtype=f)

    onsq = np.zeros((H, 4, 32), f)
    for r in range(4):
        onsq[:, r, r] = 1.0
    E4 = np.zeros((H, ngrp, H), f)
    for g in range(ngrp):
        for r in range(GS):
            E4[32 * r + (g % 2), g, 32 * r:32 * (r + 1)] = 0.5

    p = {
        "LT_h4": np.tile(np.asarray(W_se1, f).T, (4, 1)),
        "LT_At": np.repeat(np.asarray(W_se2, f).T[:, 0:4], 32, axis=1),
        "LT_Ab": np.repeat(np.asarray(W_se2, f).T[:, 4:8], 32, axis=1),
        "Bse1": np.asarray(b_se1, f).reshape(H, 1),
        "Bse2t": np.repeat(np.asarray(b_se2, f)[0:4], 32).reshape(H, 1),
        "Bse2b": np.repeat(np.asarray(b_se2, f)[4:8], 32).reshape(H, 1),
        "LT_z4": np.tile(I32, (4, 4)),
        "LT_w1z4": np.tile(np.asarray(W1, f).T, (4, 1)),
        "B1": np.asarray(b1, f).reshape(H, 1),
        "LT_t1": np.ascontiguousarray(W1G[:H]),
        "LT_b1": np.ascontiguousarray(W1G[H:]),
        "onsq4": onsq.reshape(H, H),
        "E4": E4.reshape(H, ngrp * H),
        "LT_w2c": np.tile(np.asarray(W2, f).T, (1, 4)),
        "B2r": np.tile(np.asarray(b2, f), 4).reshape(H, 1),
    }
    for k, tname, bname in ((4, "LT_t4", "LT_b4"), (3, "LT_t3", "LT_b3"), (2, "LT_t2", "LT_b2")):
        scaled = np.tile(Gflat * f(1.0 / k), (1, 4))
        p[tname] = np.ascontiguousarray(scaled[:H])
        p[bname] = np.ascontiguousarray(scaled[H:])
    for name, (shape, dt) in _PARAM_SHAPES.items():
        assert list(p[name].shape) == shape, (name, p[name].shape, shape)
        if dt == BF16:
            p[name] = p[name].astype(bf)
        else:
            p[name] = np.ascontiguousarray(p[name], f)
    return p


def _run(z, G, W_se1, b_se1, W_se2, b_se2, W1, b1, W2, b2, trace=False, **trace_kw):
    import ml_dtypes
    z = np.asarray(z, np.float32)
    nt = BC // BT
    # b_se1/b1 go through ACT bias slots either way; only b_se2/b2 change
    # the instruction count.
    zero_bias = (float(np.abs(np.asarray(b_se2)).max()) == 0.0
                 and float(np.abs(np.asarray(b2)).max()) == 0.0)
    params = _host_params(G, W_se1, b_se1, W_se2, b_se2, W1, b1, W2, b2, nt)

    zT = np.ascontiguousarray(
        z.reshape(NCORES, BC, D).transpose(0, 2, 1)
    ).astype(ml_dtypes.bfloat16)

    nc = _build_program(BC, zero_bias)
    in_maps = [{"zT": zT[c], **params} for c in range(NCORES)]
    res = run_bass_kernel_spmd(nc, in_maps, list(range(NCORES)), trace=trace, **trace_kw)

    # out4 per core: [128, ngrp*BT]; row 32r+j, col g*BT+b -> sample (4g+r)*BT+b, feature j
    ngrp = (BC // BT) // GS
    out4 = np.stack([res.results[c]["out4"] for c in range(NCORES)])
    out = (out4.reshape(NCORES, GS, D, ngrp, BT)
           .transpose(0, 3, 1, 4, 2)          # core, g, r, b, j
           .reshape(B, D))
    return np.ascontiguousarray(out.astype(np.float32)), res


def kernel(z, G, W_se1, b_se1, W_se2, b_se2, W1, b1, W2, b2):
    out, _ = _run(z, G, W_se1, b_se1, W_se2, b_se2, W1, b1, W2, b2, trace=False)
    return out


if __name__ == "__main__":
    rng = np.random.default_rng(0)
    inputs = {
        "z": rng.standard_normal((B, D), dtype=np.float32),
        "G": (rng.standard_normal((NG, D, D)) * 0.1).astype(np.float32),
        "W_se1": (rng.standard_normal((H, D)) / np.sqrt(D)).astype(np.float32),
        "b_se1": np.zeros(H, np.float32),
        "W_se2": (rng.standard_normal((NG, H)) / np.sqrt(H)).astype(np.float32),
        "b_se2": np.zeros(NG, np.float32),
        "W1": (rng.standard_normal((H, D)) * 0.01).astype(np.float32),
        "b1": np.zeros(H, np.float32),
        "W2": (rng.standard_normal((D, H)) * 0.01).astype(np.float32),
        "b2": np.zeros(D, np.float32),
    }
    out = kernel(**inputs)
    print("kernel output", out.shape, out.dtype, float(np.abs(out).max()))
